# revision 11
# baseline (speedup 1.0000x reference)
"""Multi-head attention with RoPE on 8 Trainium2 NeuronCores.

Strategy: tensor-parallel over heads (16 heads / 8 cores = 2 heads per
core). Each core computes its 2 heads' q/k/v projections, RoPE, full
(non-causal) softmax attention, and a partial output projection over its
128-dim slice of the attention output; the host sums the 8 partial
outputs and adds the output bias.

Layout notes (per core, hidden=1024, S=seq, D=64 head dim):
 - x is pre-transposed on the host to xT [1024, B*S] so projection
   matmuls contract over the hidden dim on SBUF partitions with
   contiguous DMA.
 - q/k are produced directly in [e=128, t] (head-major) layout; RoPE is
   folded into duplicated/permuted projection weights (QA/QB) combined
   with host-precomputed cos/sin tables C1/C2, so no on-device
   permutation is needed:  q_roped = (x@WqA.T + bqA) * C1 + (x@WqB.T + bqB) * C2
 - scores are computed transposed (S^T [tk, tq]) so exp(S^T) feeds the
   PV matmul directly with no transposes anywhere.  Softmax max-
   subtraction is skipped (scores are O(1) here), and the denominator is
   obtained by appending a ones column to V (M=65 matmul).  The
   reciprocal is broadcast across partitions with a tiny selector
   matmul, and normalization commutes with the (per-token) out-proj.
 - matmul operands are bf16 (full-rate streaming, half the DMA and
   weight-load traffic); all accumulation stays fp32 in PSUM and the
   bias/rope/softmax arithmetic runs in fp32 on the DVE/ACT before the
   bf16 store.
"""

import sys

for _p in ("/opt/trn_rl_repo",):
    if _p not in sys.path:
        sys.path.append(_p)

import numpy as np

import concourse.bacc as bacc
import concourse.bass as bass
import concourse.mybir as mybir
import concourse.tile as tile
from concourse.bass_utils import run_bass_kernel_spmd

F32 = mybir.dt.float32
F32R = mybir.dt.float32r
BF16 = mybir.dt.bfloat16
MMDT = BF16
ALU = mybir.AluOpType

HIDDEN = 1024
HEADS = 16
D = 64
ROPE_BASE = 10000.0
NCORES = 8
HPC = HEADS // NCORES  # heads per core = 2
EPC = HPC * D  # out dims per core = 128
B_FULL, S_FULL = 4, 2048

TQ = 512  # query-chunk width
TK = 128  # key-block width


import ml_dtypes


def to_mm(a):
    """Convert an fp32 array to the matmul operand dtype (bf16, RNE)."""
    return np.ascontiguousarray(np.asarray(a, np.float32).astype(ml_dtypes.bfloat16))


def build_nc(B, S):
    """Build the per-core Bass program (SPMD: all cores run this)."""
    nc = bacc.Bacc(None, target_bir_lowering=False)
    T = B * S
    NTQ = S // TQ  # tq chunks per batch
    NTK = S // TK  # tk blocks per batch
    DC = HIDDEN // 128  # contraction chunks

    xt_d = nc.dram_tensor("xt", [HIDDEN, T], MMDT, kind="ExternalInput")
    wqa_d = nc.dram_tensor("wqa", [HIDDEN, EPC], MMDT, kind="ExternalInput")
    wqb_d = nc.dram_tensor("wqb", [HIDDEN, EPC], MMDT, kind="ExternalInput")
    wka_d = nc.dram_tensor("wka", [HIDDEN, EPC], MMDT, kind="ExternalInput")
    wkb_d = nc.dram_tensor("wkb", [HIDDEN, EPC], MMDT, kind="ExternalInput")
    wv_d = nc.dram_tensor("wv", [HIDDEN, EPC], MMDT, kind="ExternalInput")
    wo_d = nc.dram_tensor("wo", [EPC, HIDDEN], MMDT, kind="ExternalInput")
    bq4_d = nc.dram_tensor("bq4", [EPC, 4], F32, kind="ExternalInput")
    bvb_d = nc.dram_tensor("bvb", [EPC, EPC], F32, kind="ExternalInput")
    c1_d = nc.dram_tensor("c1", [EPC, S], F32, kind="ExternalInput")
    c2_d = nc.dram_tensor("c2", [EPC, S], F32, kind="ExternalInput")
    sel2_d = nc.dram_tensor("sel2", [64, EPC], MMDT, kind="ExternalInput")
    yt_d = nc.dram_tensor("yt", [HIDDEN, T], F32, kind="ExternalOutput")

    with tile.TileContext(nc) as tc:
        with (
            tc.tile_pool(name="const", bufs=1) as cpool,
            tc.tile_pool(name="xin", bufs=2) as xpool,
            tc.tile_pool(name="qk", bufs=2) as qkpool,
            tc.tile_pool(name="vsb", bufs=2) as vpool,
            tc.tile_pool(name="esb", bufs=6) as epool,
            tc.tile_pool(name="work", bufs=4) as wpool,
            tc.tile_pool(name="yout", bufs=4) as ypool,
            tc.tile_pool(name="psA", bufs=1, space="PSUM") as psA,
            tc.tile_pool(name="psP", bufs=1, space="PSUM") as psP,
            tc.tile_pool(name="psO", bufs=1, space="PSUM") as psO,
            tc.tile_pool(name="psY", bufs=1, space="PSUM") as psY,
        ):
            # ---- constants (resident all kernel) ----
            def _ldw(dram):
                t = cpool.tile([128, DC, EPC], MMDT, tag=dram.name + "_sb")
                nc.sync.dma_start(t[:], dram[:, :].rearrange("(dc p) m -> p dc m", p=128))
                return t

            wqa = _ldw(wqa_d)
            wqb = _ldw(wqb_d)
            wka = _ldw(wka_d)
            wkb = _ldw(wkb_d)
            wv = _ldw(wv_d)
            wo = cpool.tile([EPC, HIDDEN], MMDT, tag="wo_sb")
            nc.sync.dma_start(wo[:], wo_d[:, :])
            bq4 = cpool.tile([EPC, 4], F32, tag="bq4_sb")
            nc.sync.dma_start(bq4[:], bq4_d[:, :])
            bvb = cpool.tile([EPC, EPC], F32, tag="bvb_sb")
            nc.sync.dma_start(bvb[:], bvb_d[:, :])
            c1 = cpool.tile([EPC, S], F32, tag="c1_sb")
            nc.sync.dma_start(c1[:], c1_d[:, :])
            c2 = cpool.tile([EPC, S], F32, tag="c2_sb")
            nc.sync.dma_start(c2[:], c2_d[:, :])
            sel2 = cpool.tile([64, EPC], MMDT, tag="sel2_sb")
            nc.sync.dma_start(sel2[:], sel2_d[:, :])
            # persistent rowsum staging tile; zeroed once so the unused
            # partitions contribute 0 (not garbage) to the selector matmul
            r2 = cpool.tile([64, TQ], MMDT, tag="r2_sb")
            nc.vector.memset(r2[:], 0.0)

            for b in range(B):
                # ================= projections + rope =================
                q_sb = qkpool.tile([EPC, S], MMDT, tag="q")
                k_sb = qkpool.tile([EPC, S], MMDT, tag="k")
                v_sb = vpool.tile([128, NTK, 2 * D + 2], MMDT, tag="v")
                # ones columns for the softmax-denominator matmul
                nc.vector.memset(v_sb[:, :, D : D + 1], 1.0)
                nc.vector.memset(v_sb[:, :, 2 * D + 1 : 2 * D + 2], 1.0)

                for t4 in range(NTQ):
                    tlo = b * S + t4 * TQ
                    ts_ = slice(t4 * TQ, (t4 + 1) * TQ)
                    xt_t = xpool.tile([128, DC, TQ], MMDT, tag="xt")
                    nc.sync.dma_start(
                        xt_t[:],
                        xt_d[:, tlo : tlo + TQ].rearrange("(dc p) t -> p dc t", p=128),
                    )

                    # q/k projections (duplicated rope weights, 2 halves);
                    # single-bank PSUM tiles so projections of the next batch
                    # can trickle in while attention owns the score banks
                    for wa, wb, bi, dst in (
                        (wqa, wqb, 0, q_sb),
                        (wka, wkb, 2, k_sb),
                    ):
                        pa = psP.tile([128, TQ], F32, tag="pj")
                        for dc in range(DC):
                            nc.tensor.matmul(
                                pa[:], wa[:, dc], xt_t[:, dc],
                                start=(dc == 0), stop=(dc == DC - 1),
                            )
                        t1 = wpool.tile([EPC, TQ], F32, tag="rope")
                        nc.vector.scalar_tensor_tensor(
                            t1[:], pa[:], bq4[:, bi : bi + 1], c1[:, ts_],
                            ALU.add, ALU.mult,
                        )
                        pb = psP.tile([128, TQ], F32, tag="pj")
                        for dc in range(DC):
                            nc.tensor.matmul(
                                pb[:], wb[:, dc], xt_t[:, dc],
                                start=(dc == 0), stop=(dc == DC - 1),
                            )
                        t2 = wpool.tile([EPC, TQ], F32, tag="rope")
                        nc.vector.scalar_tensor_tensor(
                            t2[:], pb[:], bq4[:, bi + 1 : bi + 2],
                            c2[:, ts_], ALU.add, ALU.mult,
                        )
                        nc.vector.tensor_add(dst[:, ts_], t1[:], t2[:])

                    # v projection (natural [t, e] layout)
                    vp = psP.tile([128, TQ], F32, tag="pj")
                    for i in range(TQ // TK):
                        cs = slice(i * TK, (i + 1) * TK)
                        for dc in range(DC):
                            nc.tensor.matmul(
                                vp[:, cs],
                                xt_t[:, dc, cs],
                                wv[:, dc],
                                start=(dc == 0),
                                stop=(dc == DC - 1),
                            )
                        blk = t4 * (TQ // TK) + i
                        nc.vector.tensor_add(
                            v_sb[:, blk, 0:D], vp[:, i * TK : i * TK + D], bvb[:, 0:D]
                        )
                        nc.vector.tensor_add(
                            v_sb[:, blk, D + 1 : 2 * D + 1],
                            vp[:, i * TK + D : (i + 1) * TK],
                            bvb[:, D:EPC],
                        )

                # ================= attention =================
                for tq in range(NTQ):
                    qs = slice(tq * TQ, (tq + 1) * TQ)
                    o0 = psO.tile([128, TQ], F32, tag="o0")
                    o1 = psO.tile([128, TQ], F32, tag="o1")
                    for tkp in range(NTK // 2):
                        # two tk-blocks per PSUM tile -> one big exp op
                        sp = psA.tile([128, 4 * TQ], F32, tag="big")
                        for u in (0, 1):
                            ks = slice((2 * tkp + u) * TK, (2 * tkp + u + 1) * TK)
                            off = u * 2 * TQ
                            nc.tensor.matmul(
                                sp[:, off : off + TQ], k_sb[0:D, ks], q_sb[0:D, qs]
                            )
                            nc.tensor.matmul(
                                sp[:, off + TQ : off + 2 * TQ],
                                k_sb[D:EPC, ks],
                                q_sb[D:EPC, qs],
                            )
                        e_sb = epool.tile([128, 4 * TQ], MMDT, tag="e")
                        nc.scalar.activation(
                            e_sb[:], sp[:], mybir.ActivationFunctionType.Exp
                        )
                        for u in (0, 1):
                            tk = 2 * tkp + u
                            off = u * 2 * TQ
                            st = dict(start=(tk == 0), stop=(tk == NTK - 1))
                            nc.tensor.matmul(
                                o0[0 : D + 1, :],
                                v_sb[:, tk, 0 : D + 1],
                                e_sb[:, off : off + TQ],
                                **st,
                            )
                            nc.tensor.matmul(
                                o1[0 : D + 1, :],
                                v_sb[:, tk, D + 1 : 2 * D + 2],
                                e_sb[:, off + TQ : off + 2 * TQ],
                                **st,
                            )
                    # softmax normalization: broadcast rowsums, reciprocal,
                    # multiply (normalization commutes with out-proj).
                    nc.vector.tensor_copy(r2[0:1, :], o0[D : D + 1, :])
                    nc.vector.tensor_copy(r2[32:33, :], o1[D : D + 1, :])
                    bp = psY.tile([128, TQ], F32, tag="y")
                    nc.tensor.matmul(bp[:], sel2[:], r2[:])
                    rec = wpool.tile([128, TQ], F32, tag="rec")
                    nc.vector.reciprocal_approx_fast(rec[:], bp[:])
                    o_sb = wpool.tile([128, TQ], MMDT, tag="osb")
                    nc.vector.tensor_tensor(
                        o_sb[0:D, :], o0[0:D, :], rec[0:D, :], ALU.mult
                    )
                    nc.vector.tensor_tensor(
                        o_sb[D:EPC, :], o1[0:D, :], rec[D:EPC, :], ALU.mult
                    )
                    # partial out-projection for this core's 128 dims
                    for eb in range(HIDDEN // 128):
                        yp = psY.tile([128, TQ], F32, tag="y")
                        nc.tensor.matmul(
                            yp[:], wo[:, eb * 128 : (eb + 1) * 128], o_sb[:]
                        )
                        y_sb = ypool.tile([128, TQ], F32, tag="ysb")
                        nc.vector.tensor_copy(y_sb[:], yp[:])
                        nc.sync.dma_start(
                            yt_d[
                                eb * 128 : (eb + 1) * 128,
                                b * S + tq * TQ : b * S + (tq + 1) * TQ,
                            ],
                            y_sb[:],
                        )

    nc.compile()
    return nc


def host_prep(x, Wq, bq, Wk, bk, Wv, bv, Wo, bo):
    """Build the 8 per-core input maps from the full-size inputs."""
    B, S, _ = x.shape
    T = B * S
    xt = to_mm(x.reshape(T, HIDDEN).T)

    # rope tables (head-dim layout, duplicated for the 2 local heads)
    j = np.arange(D // 2)
    inv_freq = 1.0 / (ROPE_BASE ** (2 * j / D))
    t = np.arange(S, dtype=np.float64)
    fr = np.outer(t, inv_freq)  # [S, 32]
    cf = np.cos(fr).T  # [32, S]
    sf = np.sin(fr).T
    c1h = np.concatenate([cf, sf], 0)  # [64, S]
    c2h = np.concatenate([-sf, cf], 0)
    c1 = np.ascontiguousarray(np.tile(c1h, (HPC, 1)).astype(np.float32))
    c2 = np.ascontiguousarray(np.tile(c2h, (HPC, 1)).astype(np.float32))

    # permutations within each local head's 64 output dims
    jj = np.arange(D)
    pa_h = 2 * (jj % (D // 2))  # even source rows, duplicated
    pb_h = pa_h + 1
    pa = np.concatenate([h * D + pa_h for h in range(HPC)])
    pb = np.concatenate([h * D + pb_h for h in range(HPC)])

    sel2 = np.zeros((64, EPC), np.float32)
    sel2[0, 0:D] = 1.0
    sel2[32, D : 2 * D] = 1.0

    scale = 1.0 / np.sqrt(D)
    in_maps = []
    for c in range(NCORES):
        rows = slice(c * EPC, (c + 1) * EPC)
        Rq, Rk, Rv = Wq[rows], Wk[rows], Wv[rows]
        bqc, bkc, bvc = bq[rows], bk[rows], bv[rows]
        m = {
            "xt": xt,
            "wqa": to_mm((Rq[pa] * scale).T),
            "wqb": to_mm((Rq[pb] * scale).T),
            "wka": to_mm(Rk[pa].T),
            "wkb": to_mm(Rk[pb].T),
            "wv": to_mm(Rv.T),
            "wo": to_mm(Wo[:, rows].T),
            "bq4": np.ascontiguousarray(
                np.stack(
                    [bqc[pa] * scale, bqc[pb] * scale, bkc[pa], bkc[pb]], 1
                ).astype(np.float32)
            ),
            "bvb": np.ascontiguousarray(
                np.tile(bvc[None, :], (EPC, 1)).astype(np.float32)
            ),
            "c1": c1,
            "c2": c2,
            "sel2": to_mm(sel2),
        }
        in_maps.append(m)
    return in_maps


_NC_CACHE = {}


def _get_nc(B, S):
    key = (B, S)
    if key not in _NC_CACHE:
        _NC_CACHE[key] = build_nc(B, S)
    return _NC_CACHE[key]


def run_cores(in_maps, B, S, trace=False):
    nc = _get_nc(B, S)
    return run_bass_kernel_spmd(
        nc, in_maps, core_ids=list(range(NCORES)), trace=trace
    )


def gather(results, bo, B, S):
    acc = results[0]["yt"].astype(np.float32)
    for c in range(1, NCORES):
        acc = acc + results[c]["yt"]
    y = acc.T + bo[None, :]
    return np.ascontiguousarray(y.reshape(B, S, HIDDEN).astype(np.float32))


def kernel(x, Wq, bq, Wk, bk, Wv, bv, Wo, bo):
    x = np.asarray(x, np.float32)
    B, S, _ = x.shape
    in_maps = host_prep(
        x,
        np.asarray(Wq, np.float32), np.asarray(bq, np.float32),
        np.asarray(Wk, np.float32), np.asarray(bk, np.float32),
        np.asarray(Wv, np.float32), np.asarray(bv, np.float32),
        np.asarray(Wo, np.float32), np.asarray(bo, np.float32),
    )
    res = run_cores(in_maps, B, S, trace=False)
    return gather(res.results, np.asarray(bo, np.float32), B, S)


# revision 13
# speedup vs baseline: 1.2192x; 1.2192x over previous
"""Multi-head attention with RoPE on 8 Trainium2 NeuronCores.

Strategy: tensor-parallel over heads (16 heads / 8 cores = 2 heads per
core). Each core computes its 2 heads' q/k/v projections, RoPE, full
(non-causal) softmax attention, and a partial output projection over its
128-dim slice of the attention output; the host sums the 8 partial
outputs and adds the output bias.

Layout notes (per core, hidden=1024, S=seq, D=64 head dim):
 - x is pre-transposed on the host to xT [1024, B*S] so projection
   matmuls contract over the hidden dim on SBUF partitions with
   contiguous DMA.
 - q/k are produced directly in [e=128, t] (head-major) layout; RoPE is
   folded into duplicated/permuted projection weights (QA/QB) combined
   with host-precomputed cos/sin tables C1/C2, so no on-device
   permutation is needed:  q_roped = (x@WqA.T + bqA) * C1 + (x@WqB.T + bqB) * C2
 - scores are computed transposed (S^T [tk, tq]) so exp(S^T) feeds the
   PV matmul directly with no transposes anywhere.  Softmax max-
   subtraction is skipped (scores are O(1) here), and the denominator is
   obtained by appending a ones column to V (M=65 matmul).  The
   reciprocal is broadcast across partitions with a tiny selector
   matmul, and normalization commutes with the (per-token) out-proj.
 - matmul operands are bf16 (full-rate streaming, half the DMA and
   weight-load traffic); all accumulation stays fp32 in PSUM and the
   bias/rope/softmax arithmetic runs in fp32 on the DVE/ACT before the
   bf16 store.
"""

import sys

for _p in ("/opt/trn_rl_repo",):
    if _p not in sys.path:
        sys.path.append(_p)

import numpy as np

import concourse.bacc as bacc
import concourse.bass as bass
import concourse.mybir as mybir
import concourse.tile as tile
from concourse.bass_utils import run_bass_kernel_spmd

F32 = mybir.dt.float32
F32R = mybir.dt.float32r
BF16 = mybir.dt.bfloat16
MMDT = BF16
ALU = mybir.AluOpType

HIDDEN = 1024
HEADS = 16
D = 64
ROPE_BASE = 10000.0
NCORES = 8
HPC = HEADS // NCORES  # heads per core = 2
EPC = HPC * D  # out dims per core = 128
B_FULL, S_FULL = 4, 2048

TQ = 512  # query-chunk width
TK = 128  # key-block width


import ml_dtypes


def to_mm(a):
    """Convert an fp32 array to the matmul operand dtype (bf16, RNE)."""
    return np.ascontiguousarray(np.asarray(a, np.float32).astype(ml_dtypes.bfloat16))


def build_nc(B, S):
    """Build the per-core Bass program (SPMD: all cores run this)."""
    nc = bacc.Bacc(None, target_bir_lowering=False)
    T = B * S
    NTQ = S // TQ  # tq chunks per batch
    NTK = S // TK  # tk blocks per batch
    DC = HIDDEN // 128  # contraction chunks

    xt_d = nc.dram_tensor("xt", [HIDDEN, T], MMDT, kind="ExternalInput")
    wq_d = nc.dram_tensor("wq", [HIDDEN, EPC], MMDT, kind="ExternalInput")
    wk_d = nc.dram_tensor("wk", [HIDDEN, EPC], MMDT, kind="ExternalInput")
    psw_d = nc.dram_tensor("psw", [EPC, EPC], MMDT, kind="ExternalInput")
    wv_d = nc.dram_tensor("wv", [HIDDEN, EPC], MMDT, kind="ExternalInput")
    wo_d = nc.dram_tensor("wo", [EPC, HIDDEN], MMDT, kind="ExternalInput")
    bq4_d = nc.dram_tensor("bq4", [EPC, 4], F32, kind="ExternalInput")
    bvb_d = nc.dram_tensor("bvb", [EPC, EPC], F32, kind="ExternalInput")
    c1_d = nc.dram_tensor("c1", [EPC, S], F32, kind="ExternalInput")
    c2_d = nc.dram_tensor("c2", [EPC, S], F32, kind="ExternalInput")
    sel2_d = nc.dram_tensor("sel2", [64, EPC], MMDT, kind="ExternalInput")
    yt_d = nc.dram_tensor("yt", [HIDDEN, T], F32, kind="ExternalOutput")

    with tile.TileContext(nc) as tc:
        with (
            tc.tile_pool(name="const", bufs=1) as cpool,
            tc.tile_pool(name="xin", bufs=2) as xpool,
            tc.tile_pool(name="qk", bufs=2) as qkpool,
            tc.tile_pool(name="vsb", bufs=2) as vpool,
            tc.tile_pool(name="esb", bufs=6) as epool,
            tc.tile_pool(name="work", bufs=4) as wpool,
            tc.tile_pool(name="yout", bufs=4) as ypool,
            tc.tile_pool(name="psA", bufs=2, space="PSUM") as psA,
            tc.tile_pool(name="psP", bufs=1, space="PSUM") as psP,
            tc.tile_pool(name="psO", bufs=1, space="PSUM") as psO,
            tc.tile_pool(name="psY", bufs=1, space="PSUM") as psY,
        ):
            # ---- constants (resident all kernel) ----
            def _ldw(dram):
                t = cpool.tile([128, DC, EPC], MMDT, tag=dram.name + "_sb")
                nc.sync.dma_start(t[:], dram[:, :].rearrange("(dc p) m -> p dc m", p=128))
                return t

            wq = _ldw(wq_d)
            wk = _ldw(wk_d)
            wv = _ldw(wv_d)
            psw = cpool.tile([EPC, EPC], MMDT, tag="psw_sb")
            nc.sync.dma_start(psw[:], psw_d[:, :])
            wo = cpool.tile([EPC, HIDDEN], MMDT, tag="wo_sb")
            nc.sync.dma_start(wo[:], wo_d[:, :])
            bq4 = cpool.tile([EPC, 4], F32, tag="bq4_sb")
            nc.sync.dma_start(bq4[:], bq4_d[:, :])
            bvb = cpool.tile([EPC, EPC], F32, tag="bvb_sb")
            nc.sync.dma_start(bvb[:], bvb_d[:, :])
            c1 = cpool.tile([EPC, S], F32, tag="c1_sb")
            nc.sync.dma_start(c1[:], c1_d[:, :])
            c2 = cpool.tile([EPC, S], F32, tag="c2_sb")
            nc.sync.dma_start(c2[:], c2_d[:, :])
            sel2 = cpool.tile([64, EPC], MMDT, tag="sel2_sb")
            nc.sync.dma_start(sel2[:], sel2_d[:, :])
            # persistent rowsum staging tile; zeroed once so the unused
            # partitions contribute 0 (not garbage) to the selector matmul
            r2 = cpool.tile([64, TQ], MMDT, tag="r2_sb")
            nc.vector.memset(r2[:], 0.0)

            for b in range(B):
                # ================= projections + rope =================
                q_sb = qkpool.tile([EPC, S], MMDT, tag="q")
                k_sb = qkpool.tile([EPC, S], MMDT, tag="k")
                v_sb = vpool.tile([128, NTK, 2 * D + 2], MMDT, tag="v")
                # ones columns for the softmax-denominator matmul
                nc.vector.memset(v_sb[:, :, D : D + 1], 1.0)
                nc.vector.memset(v_sb[:, :, 2 * D + 1 : 2 * D + 2], 1.0)

                for t4 in range(NTQ):
                    tlo = b * S + t4 * TQ
                    ts_ = slice(t4 * TQ, (t4 + 1) * TQ)
                    xt_t = xpool.tile([128, DC, TQ], MMDT, tag="xt")
                    nc.sync.dma_start(
                        xt_t[:],
                        xt_d[:, tlo : tlo + TQ].rearrange("(dc p) t -> p dc t", p=128),
                    )

                    # q/k projections in interleaved head layout (rope
                    # pairs are adjacent rows; the dot product is invariant
                    # to the within-head order, so no de-interleave needed).
                    # q' = (q + b) * C1 + (swap(q) + swap(b)) * C2, where
                    # swap exchanges adjacent partitions via a tiny matmul.
                    for wa, bi, dst in (
                        (wq, 0, q_sb),
                        (wk, 2, k_sb),
                    ):
                        pa = psP.tile([128, TQ], F32, tag="pj")
                        for dc in range(DC):
                            nc.tensor.matmul(
                                pa[:], wa[:, dc], xt_t[:, dc],
                                start=(dc == 0), stop=(dc == DC - 1),
                            )
                        praw = wpool.tile([EPC, TQ], MMDT, tag="praw")
                        nc.vector.tensor_copy(praw[:], pa[:])
                        t1 = wpool.tile([EPC, TQ], F32, tag="rope")
                        nc.vector.scalar_tensor_tensor(
                            t1[:], pa[:], bq4[:, bi : bi + 1], c1[:, ts_],
                            ALU.add, ALU.mult,
                        )
                        pb = psP.tile([128, TQ], F32, tag="pj")
                        nc.tensor.matmul(pb[:], psw[:], praw[:])
                        t2 = wpool.tile([EPC, TQ], F32, tag="rope")
                        nc.vector.scalar_tensor_tensor(
                            t2[:], pb[:], bq4[:, bi + 1 : bi + 2],
                            c2[:, ts_], ALU.add, ALU.mult,
                        )
                        nc.vector.tensor_add(dst[:, ts_], t1[:], t2[:])

                    # v projection (natural [t, e] layout)
                    vp = psP.tile([128, TQ], F32, tag="pj")
                    for i in range(TQ // TK):
                        cs = slice(i * TK, (i + 1) * TK)
                        for dc in range(DC):
                            nc.tensor.matmul(
                                vp[:, cs],
                                xt_t[:, dc, cs],
                                wv[:, dc],
                                start=(dc == 0),
                                stop=(dc == DC - 1),
                            )
                        blk = t4 * (TQ // TK) + i
                        nc.vector.tensor_add(
                            v_sb[:, blk, 0:D], vp[:, i * TK : i * TK + D], bvb[:, 0:D]
                        )
                        nc.vector.tensor_add(
                            v_sb[:, blk, D + 1 : 2 * D + 1],
                            vp[:, i * TK + D : (i + 1) * TK],
                            bvb[:, D:EPC],
                        )

                # ================= attention =================
                for tq in range(NTQ):
                    qs = slice(tq * TQ, (tq + 1) * TQ)
                    o0 = psO.tile([128, TQ], F32, tag="o0")
                    o1 = psO.tile([128, TQ], F32, tag="o1")
                    for tk in range(NTK):
                        ks = slice(tk * TK, (tk + 1) * TK)
                        sp = psA.tile([128, 2 * TQ], F32, tag="big")
                        nc.tensor.matmul(
                            sp[:, 0:TQ], k_sb[0:D, ks], q_sb[0:D, qs]
                        )
                        nc.tensor.matmul(
                            sp[:, TQ : 2 * TQ],
                            k_sb[D:EPC, ks],
                            q_sb[D:EPC, qs],
                        )
                        e_sb = epool.tile([128, 2 * TQ], MMDT, tag="e")
                        nc.scalar.activation(
                            e_sb[:], sp[:], mybir.ActivationFunctionType.Exp
                        )
                        st = dict(start=(tk == 0), stop=(tk == NTK - 1))
                        nc.tensor.matmul(
                            o0[0 : D + 1, :],
                            v_sb[:, tk, 0 : D + 1],
                            e_sb[:, 0:TQ],
                            **st,
                        )
                        nc.tensor.matmul(
                            o1[0 : D + 1, :],
                            v_sb[:, tk, D + 1 : 2 * D + 2],
                            e_sb[:, TQ : 2 * TQ],
                            **st,
                        )
                    # softmax normalization: broadcast rowsums, reciprocal,
                    # multiply (normalization commutes with out-proj).
                    nc.vector.tensor_copy(r2[0:1, :], o0[D : D + 1, :])
                    nc.vector.tensor_copy(r2[32:33, :], o1[D : D + 1, :])
                    bp = psY.tile([128, TQ], F32, tag="y")
                    nc.tensor.matmul(bp[:], sel2[:], r2[:])
                    rec = wpool.tile([128, TQ], F32, tag="rec")
                    nc.vector.reciprocal_approx_fast(rec[:], bp[:])
                    o_sb = wpool.tile([128, TQ], MMDT, tag="osb")
                    nc.vector.tensor_tensor(
                        o_sb[0:D, :], o0[0:D, :], rec[0:D, :], ALU.mult
                    )
                    nc.vector.tensor_tensor(
                        o_sb[D:EPC, :], o1[0:D, :], rec[D:EPC, :], ALU.mult
                    )
                    # partial out-projection for this core's 128 dims
                    for eb in range(HIDDEN // 128):
                        yp = psY.tile([128, TQ], F32, tag="y")
                        nc.tensor.matmul(
                            yp[:], wo[:, eb * 128 : (eb + 1) * 128], o_sb[:]
                        )
                        y_sb = ypool.tile([128, TQ], F32, tag="ysb")
                        nc.vector.tensor_copy(y_sb[:], yp[:])
                        nc.sync.dma_start(
                            yt_d[
                                eb * 128 : (eb + 1) * 128,
                                b * S + tq * TQ : b * S + (tq + 1) * TQ,
                            ],
                            y_sb[:],
                        )

    nc.compile()
    return nc


def host_prep(x, Wq, bq, Wk, bk, Wv, bv, Wo, bo):
    """Build the 8 per-core input maps from the full-size inputs."""
    B, S, _ = x.shape
    T = B * S
    xt = to_mm(x.reshape(T, HIDDEN).T)

    # rope tables in INTERLEAVED head layout: row 2j and 2j+1 share
    # frequency j.  q'[2j] = q[2j] c_j - q[2j+1] s_j ;
    # q'[2j+1] = q[2j] s_j + q[2j+1] c_j.  With swap() exchanging rows
    # 2j <-> 2j+1:  q' = q * C1 + swap(q) * C2,
    # C1[2j]=C1[2j+1]=c_j, C2[2j]=-s_j, C2[2j+1]=+s_j.
    j = np.arange(D // 2)
    inv_freq = 1.0 / (ROPE_BASE ** (2 * j / D))
    t = np.arange(S, dtype=np.float64)
    fr = np.outer(t, inv_freq)  # [S, 32]
    cf = np.cos(fr).T  # [32, S]
    sf = np.sin(fr).T
    c1h = np.repeat(cf, 2, axis=0)  # [64, S]
    c2h = np.empty((D, S))
    c2h[0::2] = -sf
    c2h[1::2] = sf
    c1 = np.ascontiguousarray(np.tile(c1h, (HPC, 1)).astype(np.float32))
    c2 = np.ascontiguousarray(np.tile(c2h, (HPC, 1)).astype(np.float32))

    # adjacent-pair swap permutation (within the 128 local rows)
    swp = np.arange(EPC)
    swp = swp ^ 1  # 2j <-> 2j+1
    psw = np.zeros((EPC, EPC), np.float32)
    psw[swp, np.arange(EPC)] = 1.0

    sel2 = np.zeros((64, EPC), np.float32)
    sel2[0, 0:D] = 1.0
    sel2[32, D : 2 * D] = 1.0

    scale = 1.0 / np.sqrt(D)
    in_maps = []
    for c in range(NCORES):
        rows = slice(c * EPC, (c + 1) * EPC)
        Rq, Rk, Rv = Wq[rows], Wk[rows], Wv[rows]
        bqc, bkc, bvc = bq[rows], bk[rows], bv[rows]
        m = {
            "xt": xt,
            "wq": to_mm((Rq * scale).T),
            "wk": to_mm(Rk.T),
            "wv": to_mm(Rv.T),
            "wo": to_mm(Wo[:, rows].T),
            "psw": to_mm(psw),
            "bq4": np.ascontiguousarray(
                np.stack(
                    [bqc * scale, bqc[swp] * scale, bkc, bkc[swp]], 1
                ).astype(np.float32)
            ),
            "bvb": np.ascontiguousarray(
                np.tile(bvc[None, :], (EPC, 1)).astype(np.float32)
            ),
            "c1": c1,
            "c2": c2,
            "sel2": to_mm(sel2),
        }
        in_maps.append(m)
    return in_maps


_NC_CACHE = {}


def _get_nc(B, S):
    key = (B, S)
    if key not in _NC_CACHE:
        _NC_CACHE[key] = build_nc(B, S)
    return _NC_CACHE[key]


def run_cores(in_maps, B, S, trace=False):
    nc = _get_nc(B, S)
    return run_bass_kernel_spmd(
        nc, in_maps, core_ids=list(range(NCORES)), trace=trace
    )


def gather(results, bo, B, S):
    acc = results[0]["yt"].astype(np.float32)
    for c in range(1, NCORES):
        acc = acc + results[c]["yt"]
    y = acc.T + bo[None, :]
    return np.ascontiguousarray(y.reshape(B, S, HIDDEN).astype(np.float32))


def kernel(x, Wq, bq, Wk, bk, Wv, bv, Wo, bo):
    x = np.asarray(x, np.float32)
    B, S, _ = x.shape
    in_maps = host_prep(
        x,
        np.asarray(Wq, np.float32), np.asarray(bq, np.float32),
        np.asarray(Wk, np.float32), np.asarray(bk, np.float32),
        np.asarray(Wv, np.float32), np.asarray(bv, np.float32),
        np.asarray(Wo, np.float32), np.asarray(bo, np.float32),
    )
    res = run_cores(in_maps, B, S, trace=False)
    return gather(res.results, np.asarray(bo, np.float32), B, S)
